# revision 27
# baseline (speedup 1.0000x reference)
"""Trainium2 Bass kernel: FFN forward + per-sample Jacobian + Hessian.

Reference computation (per sample xi of width D=100):
    h   = gelu(xi @ W1.T + b1)            (exact erf GELU)
    y   = h @ W2.T + b2
    out[b]  = y                               (B, D)
    jac[b]  = W2 @ diag(g'(pre_b)) @ W1       (B, D, D)
    hess[b,i,j,l] = sum_k W2[i,k] g''(pre_b k) W1[k,j] W1[k,l]   (B, D, D, D)

Strategy: pure data parallel over the batch (B=128 -> 16 per core, 8 cores),
weights replicated. Per core the outputs are treated as a flat row space
r = (b, i) of BS*D = 1600 rows:
    jac_flat[r, j]   = sum_k (W2[i,k] g'(pre_b,k))  W1[k,j]
    hess_flat[r, jl] = sum_k (W2[i,k] g''(pre_b,k)) M[k, jl]
with M[k, j*D+l] = W1[k,j]*W1[k,l] built on-chip once. Rows are processed in
blocks of 128 so every matmul output, PSUM->SBUF copy, and DMA store uses all
128 partitions — a 100-partition store stream only reaches ~176 GB/s of HBM
write bandwidth while 128 partitions reach ~373 GB/s (measured), and the
512 MB hessian write is the roofline. Each 128-row block spans two samples, so
its stationary operand is built with two per-partition scales of W2.T.

The hessian matmuls run in float32r (tf32-like, 1 cycle/row; plain fp32 costs
4) — hess rel err ~1.5e-4. Forward output and jacobian stay exact fp32.
"""

import numpy as np

import concourse.bass as bass
import concourse.bacc as bacc
import concourse.mybir as mybir
from concourse.tile import TileContext
from concourse.bass_utils import run_bass_kernel_spmd

B, D = 128, 100
NCORES = 8
BS = B // NCORES  # 16 samples per core
DD = D * D
ROWS = BS * D  # 1600 output rows per core
P = 128  # rows per block
NBLK = (ROWS + P - 1) // P  # 13
RPAD = NBLK * P  # 1664: outputs padded to full blocks; host slices [:ROWS]

F32 = mybir.dt.float32
F32R = mybir.dt.float32r
ACT = mybir.ActivationFunctionType
ALU = mybir.AluOpType

INV_SQRT2 = 0.7071067811865476
INV_SQRT_2PI = 0.3989422804014327

NCHUNK = 20  # hess free-dim chunks per block
CW = 500  # chunk width (jl columns per matmul; 500*4B fits one PSUM bank)

# packed input layout: [xta | w1 | w1ta | w2ta] along the free dim
PK_X = 0
PK_W1 = PK_X + BS
PK_W1TA = PK_W1 + D
PK_W2TA = PK_W1TA + D
PK_W = PK_W2TA + D  # 316


def build_nc():
    # Bacc (not raw Bass): its finalize() runs compile() passes that split
    # multi-semaphore waits into EventSemaphore instructions (TRN2 allows at
    # most one sync wait per regular instruction).
    nc = bacc.Bacc()

    pk_d = nc.dram_tensor("pk", [D + 1, PK_W], F32, kind="ExternalInput")

    out_d = nc.dram_tensor("out", [BS, D], F32, kind="ExternalOutput")
    jac_d = nc.dram_tensor("jac", [RPAD, D], F32, kind="ExternalOutput")
    hess_d = nc.dram_tensor("hess", [RPAD, DD], F32, kind="ExternalOutput")

    with TileContext(nc) as tc:
        with (
            tc.tile_pool(name="const", bufs=1) as constp,
            tc.tile_pool(name="mpool", bufs=1) as mpool,
            tc.tile_pool(name="stage", bufs=3) as stagep,
            tc.tile_pool(name="small", bufs=2) as smallp,
            tc.tile_pool(name="spsum", bufs=2, space="PSUM") as spsum,
            tc.tile_pool(name="hpsum", bufs=6, space="PSUM") as hpsum,
        ):
            # ---- load all inputs in one DMA (single completion semaphore) ----
            pk = constp.tile([D + 1, PK_W], F32)
            nc.sync.dma_start(pk[:], pk_d[:])
            xta = pk[:, PK_X:PK_W1]  # (D+1, BS)
            w1 = pk[:D, PK_W1:PK_W1TA]  # (D, D)
            w1ta = pk[:, PK_W1TA:PK_W2TA]  # (D+1, D)
            w2ta = pk[:, PK_W2TA:PK_W]  # (D+1, D)
            w2t = pk[:D, PK_W2TA:PK_W]  # (D, D)

            # ---- forward: pre_T = [W1.T; b1].T @ [x.T; 1] ----
            pre_ps = spsum.tile([D, BS], F32, tag="sp")
            nc.tensor.matmul(pre_ps[:], w1ta, xta, start=True, stop=True)
            pre = constp.tile([D, BS], F32)
            nc.vector.tensor_copy(pre[:], pre_ps[:])

            # Phi = 0.5*(1 + erf(pre/sqrt(2)))
            erf_t = constp.tile([D, BS], F32)
            nc.scalar.activation(erf_t[:], pre[:], ACT.Erf, scale=INV_SQRT2)
            phi_t = constp.tile([D, BS], F32)
            nc.vector.tensor_scalar(
                phi_t[:], erf_t[:], 0.5, 0.5, op0=ALU.mult, op1=ALU.add
            )
            # sq = pre^2 ; ex = exp(-pre^2/2)
            sq = constp.tile([D, BS], F32)
            nc.scalar.activation(sq[:], pre[:], ACT.Square)
            ex = constp.tile([D, BS], F32)
            nc.scalar.activation(ex[:], sq[:], ACT.Exp, scale=-0.5)

            # g = pre * Phi, with ones row appended for the bias trick.
            # Compute-engine writes must start at partition 0/32/64/96, so
            # memset rows 96..100 to 1 first; the mul then overwrites 96..99.
            gta = constp.tile([D + 1, BS], F32)
            nc.vector.memset(gta[96 : D + 1, :], 1.0)
            nc.vector.tensor_mul(gta[:D, :], pre[:], phi_t[:])

            # gp = Phi + c * pre * ex          (c = 1/sqrt(2*pi))
            t1 = constp.tile([D, BS], F32)
            nc.vector.tensor_mul(t1[:], pre[:], ex[:])
            t1s = constp.tile([D, BS], F32)
            nc.vector.tensor_scalar_mul(t1s[:], t1[:], INV_SQRT_2PI)
            gp = constp.tile([D, BS], F32)
            nc.vector.tensor_add(gp[:], phi_t[:], t1s[:])

            # gpp = c * ex * (2 - sq)
            v2 = constp.tile([D, BS], F32)
            nc.vector.tensor_scalar(v2[:], sq[:], -1.0, 2.0, op0=ALU.mult, op1=ALU.add)
            gpp0 = constp.tile([D, BS], F32)
            nc.vector.tensor_mul(gpp0[:], ex[:], v2[:])
            gpp = constp.tile([D, BS], F32)
            nc.vector.tensor_scalar_mul(gpp[:], gpp0[:], INV_SQRT_2PI)

            # out = [g; 1].T @ [W2.T; b2]
            out_ps = spsum.tile([BS, D], F32, tag="sp")
            nc.tensor.matmul(out_ps[:], gta[:], w2ta, start=True, stop=True)
            out_sb = smallp.tile([BS, D], F32, tag="outsb")
            nc.vector.tensor_copy(out_sb[:], out_ps[:])
            nc.scalar.dma_start(out_d[:], out_sb[:])

            # 128-partition source for the stage-acquisition dummy copies
            dummy = constp.tile([P, 1], F32)
            nc.vector.memset(dummy[:], 0.0)

            # ---- M[k, j, l] = W1[k,j] * W1[k,l]  (rhs for hessian matmuls) ----
            # outer product via step-0 broadcast access patterns, split into
            # quarters so block 0's first chunks can start before the whole
            # 4MB table is built (deps are range-tracked)
            m_t = mpool.tile([D, D, D], F32R)
            MQ = 4
            for q in range(MQ):
                j0, j1 = q * D // MQ, (q + 1) * D // MQ
                w1_bj = bass.AP(
                    w1.tensor, w1.offset + j0, [w1.ap[0], [1, j1 - j0], [0, D]]
                )
                w1_bl = bass.AP(w1.tensor, w1.offset, [w1.ap[0], [0, j1 - j0], [1, D]])
                nc.vector.tensor_mul(m_t[:, j0:j1], w1_bj, w1_bl)

            # ---- per-block jacobian + hessian over flat rows r=(b,i) ----
            for blk in range(NBLK):
                r0 = blk * P

                def build_scaled(tile_ap, gvec):
                    # tile[:, c] = W2.T[:, i(r0+c)] * gvec[:, b(r0+c)]
                    # rows past ROWS are padding: clamped b -> garbage values,
                    # written to the padded DRAM tail and never read
                    c = 0
                    while c < P:
                        b_c, i_c = divmod(r0 + c, D)
                        b_c = min(b_c, BS - 1)
                        n_c = min(D - i_c, P - c)
                        nc.vector.tensor_scalar_mul(
                            tile_ap[:, c : c + n_c],
                            w2t[:, i_c : i_c + n_c],
                            gvec[:, b_c : b_c + 1],
                        )
                        c += n_c

                # jacobian rows r0..r0+P (exact fp32)
                jl = smallp.tile([D, P], F32, tag="jl")
                build_scaled(jl, gp)
                j_ps = spsum.tile([P, D], F32, tag="sp")
                nc.tensor.matmul(j_ps[:], jl[:], w1, start=True, stop=True)
                jstage = smallp.tile([P, D], F32, tag="jstage")
                nc.vector.tensor_copy(jstage[:], j_ps[:])
                nc.scalar.dma_start(jac_d[r0 : r0 + P, :], jstage[:])

                # hessian rows r0..r0+P (float32r)
                hl = smallp.tile([D, P], F32R, tag="hl")
                build_scaled(hl, gpp)
                stage = stagep.tile([P, NCHUNK, CW], F32)
                # dummy first touch: absorbs the buffer-reuse (hess DMA of
                # block blk-3) wait so the real copies keep short wait lists
                nc.vector.tensor_copy(stage[:, 0, :1], dummy[:])
                for c in range(NCHUNK):
                    h_ps = hpsum.tile([P, 512], F32)
                    jlo = c * (CW // D)  # start j of this chunk
                    nc.tensor.matmul(
                        h_ps[:, :CW],
                        hl[:],
                        m_t[:, jlo : jlo + CW // D, :],
                        start=True,
                        stop=True,
                    )
                    # split PSUM->SBUF copies between vector and scalar engines
                    if c % 2 == 1:
                        nc.scalar.copy(stage[:, c], h_ps[:, :CW])
                    else:
                        nc.vector.tensor_copy(stage[:, c], h_ps[:, :CW])
                nc.sync.dma_start(
                    hess_d[r0 : r0 + P, :],
                    stage[:].rearrange("p a b -> p (a b)"),
                )

    nc.finalize()
    return nc


_NC_CACHE = {}


def _get_nc():
    if "nc" not in _NC_CACHE:
        _NC_CACHE["nc"] = build_nc()
    return _NC_CACHE["nc"]


def _prep_inputs(x, W1, b1, W2, b2):
    x = np.asarray(x, dtype=np.float32)
    W1 = np.ascontiguousarray(np.asarray(W1, dtype=np.float32))
    b1 = np.asarray(b1, dtype=np.float32)
    W2 = np.asarray(W2, dtype=np.float32)
    b2 = np.asarray(b2, dtype=np.float32)

    w1pad = np.concatenate([W1, np.zeros((1, D), np.float32)], axis=0)  # (D+1, D)
    w1ta = np.concatenate([W1.T, b1[None, :]], axis=0)  # (D+1, D)
    w2ta = np.concatenate([W2.T, b2[None, :]], axis=0)  # (D+1, D)

    in_maps = []
    for c in range(NCORES):
        xs = x[c * BS : (c + 1) * BS]  # (BS, D)
        xta = np.concatenate([xs.T, np.ones((1, BS), np.float32)], axis=0)
        pk = np.ascontiguousarray(
            np.concatenate([xta, w1pad, w1ta, w2ta], axis=1)
        )  # (D+1, 316)
        in_maps.append({"pk": pk})
    return in_maps


def run(x, W1, b1, W2, b2, trace=False, **kw):
    nc = _get_nc()
    in_maps = _prep_inputs(x, W1, b1, W2, b2)
    res = run_bass_kernel_spmd(nc, in_maps, list(range(NCORES)), trace=trace, **kw)
    out = np.concatenate([r["out"] for r in res.results], axis=0)
    jac = np.concatenate(
        [r["jac"][:ROWS].reshape(BS, D, D) for r in res.results], axis=0
    )
    hess = np.concatenate(
        [r["hess"][:ROWS].reshape(BS, D, D, D) for r in res.results], axis=0
    )
    return (out, jac, hess), res


def kernel(x, W1, b1, W2, b2):
    (out, jac, hess), _ = run(x, W1, b1, W2, b2)
    return (out, jac, hess)


# revision 30
# speedup vs baseline: 1.0074x; 1.0074x over previous
"""Trainium2 Bass kernel: FFN forward + per-sample Jacobian + Hessian.

Reference computation (per sample xi of width D=100):
    h   = gelu(xi @ W1.T + b1)            (exact erf GELU)
    y   = h @ W2.T + b2
    out[b]  = y                               (B, D)
    jac[b]  = W2 @ diag(g'(pre_b)) @ W1       (B, D, D)
    hess[b,i,j,l] = sum_k W2[i,k] g''(pre_b k) W1[k,j] W1[k,l]   (B, D, D, D)

Strategy: pure data parallel over the batch (B=128 -> 16 per core, 8 cores),
weights replicated. Per core the outputs are treated as a flat row space
r = (b, i) of BS*D = 1600 rows:
    jac_flat[r, j]   = sum_k (W2[i,k] g'(pre_b,k))  W1[k,j]
    hess_flat[r, jl] = sum_k (W2[i,k] g''(pre_b,k)) M[k, jl]
with M[k, j*D+l] = W1[k,j]*W1[k,l] built on-chip once. Rows are processed in
blocks of 128 so every matmul output, PSUM->SBUF copy, and DMA store uses all
128 partitions — a 100-partition store stream only reaches ~176 GB/s of HBM
write bandwidth while 128 partitions reach ~373 GB/s (measured), and the
512 MB hessian write is the roofline. Each 128-row block spans two samples, so
its stationary operand is built with two per-partition scales of W2.T.

The hessian matmuls run in float32r (tf32-like, 1 cycle/row; plain fp32 costs
4) — hess rel err ~1.5e-4. Forward output and jacobian stay exact fp32.
"""

import numpy as np

import concourse.bass as bass
import concourse.bacc as bacc
import concourse.mybir as mybir
from concourse.tile import TileContext
from concourse.bass_utils import run_bass_kernel_spmd

B, D = 128, 100
NCORES = 8
BS = B // NCORES  # 16 samples per core
DD = D * D
ROWS = BS * D  # 1600 output rows per core
P = 128  # rows per block
NBLK = (ROWS + P - 1) // P  # 13
RPAD = NBLK * P  # 1664: outputs padded to full blocks; host slices [:ROWS]

F32 = mybir.dt.float32
F32R = mybir.dt.float32r
ACT = mybir.ActivationFunctionType
ALU = mybir.AluOpType

INV_SQRT2 = 0.7071067811865476
INV_SQRT_2PI = 0.3989422804014327

NCHUNK = 20  # hess free-dim chunks per block
CW = 500  # chunk width (jl columns per matmul; 500*4B fits one PSUM bank)

# packed input layout: [xta | w1 | w1ta | w2ta] along the free dim
PK_X = 0
PK_W1 = PK_X + BS
PK_W1TA = PK_W1 + D
PK_W2TA = PK_W1TA + D
PK_W = PK_W2TA + D  # 316


def build_nc():
    # Bacc (not raw Bass): its finalize() runs compile() passes that split
    # multi-semaphore waits into EventSemaphore instructions (TRN2 allows at
    # most one sync wait per regular instruction).
    nc = bacc.Bacc()

    pk_d = nc.dram_tensor("pk", [D + 1, PK_W], F32, kind="ExternalInput")

    out_d = nc.dram_tensor("out", [BS, D], F32, kind="ExternalOutput")
    jac_d = nc.dram_tensor("jac", [RPAD, D], F32, kind="ExternalOutput")
    hess_d = nc.dram_tensor("hess", [RPAD, DD], F32, kind="ExternalOutput")

    with TileContext(nc) as tc:
        with (
            tc.tile_pool(name="const", bufs=1) as constp,
            tc.tile_pool(name="mpool", bufs=1) as mpool,
            tc.tile_pool(name="stage", bufs=3) as stagep,
            tc.tile_pool(name="small", bufs=2) as smallp,
            tc.tile_pool(name="spsum", bufs=2, space="PSUM") as spsum,
            tc.tile_pool(name="hpsum", bufs=6, space="PSUM") as hpsum,
        ):
            # ---- load all inputs in one DMA (single completion semaphore) ----
            pk = constp.tile([D + 1, PK_W], F32)
            nc.sync.dma_start(pk[:], pk_d[:])
            xta = pk[:, PK_X:PK_W1]  # (D+1, BS)
            w1 = pk[:D, PK_W1:PK_W1TA]  # (D, D)
            w1ta = pk[:, PK_W1TA:PK_W2TA]  # (D+1, D)
            w2ta = pk[:, PK_W2TA:PK_W]  # (D+1, D)
            w2t = pk[:D, PK_W2TA:PK_W]  # (D, D)

            # prefetch both scalar-engine activation tables (erf, exp) while
            # the input DMA is still in flight; each table load costs ~1.3us
            # and would otherwise sit on the forward critical path
            warm = constp.tile([P, 1], F32)
            nc.vector.memset(warm[:], 0.0)
            nc.scalar.activation(warm[:], warm[:], ACT.Erf)
            nc.scalar.activation(warm[:], warm[:], ACT.Exp)

            # ---- forward: pre_T = [W1.T; b1].T @ [x.T; 1] ----
            pre_ps = spsum.tile([D, BS], F32, tag="sp")
            nc.tensor.matmul(pre_ps[:], w1ta, xta, start=True, stop=True)
            pre = constp.tile([D, BS], F32)
            nc.vector.tensor_copy(pre[:], pre_ps[:])

            # Phi = 0.5*(1 + erf(pre/sqrt(2)))
            erf_t = constp.tile([D, BS], F32)
            nc.scalar.activation(erf_t[:], pre[:], ACT.Erf, scale=INV_SQRT2)
            phi_t = constp.tile([D, BS], F32)
            nc.vector.tensor_scalar(
                phi_t[:], erf_t[:], 0.5, 0.5, op0=ALU.mult, op1=ALU.add
            )
            # sq = pre^2 ; ex = exp(-pre^2/2)
            sq = constp.tile([D, BS], F32)
            nc.scalar.activation(sq[:], pre[:], ACT.Square)
            ex = constp.tile([D, BS], F32)
            nc.scalar.activation(ex[:], sq[:], ACT.Exp, scale=-0.5)

            # g = pre * Phi, with ones row appended for the bias trick.
            # Compute-engine writes must start at partition 0/32/64/96, so
            # memset rows 96..100 to 1 first; the mul then overwrites 96..99.
            gta = constp.tile([D + 1, BS], F32)
            nc.vector.memset(gta[96 : D + 1, :], 1.0)
            nc.vector.tensor_mul(gta[:D, :], pre[:], phi_t[:])

            # gp = Phi + c * pre * ex          (c = 1/sqrt(2*pi))
            t1 = constp.tile([D, BS], F32)
            nc.vector.tensor_mul(t1[:], pre[:], ex[:])
            t1s = constp.tile([D, BS], F32)
            nc.vector.tensor_scalar_mul(t1s[:], t1[:], INV_SQRT_2PI)
            gp = constp.tile([D, BS], F32)
            nc.vector.tensor_add(gp[:], phi_t[:], t1s[:])

            # gpp = c * ex * (2 - sq)
            v2 = constp.tile([D, BS], F32)
            nc.vector.tensor_scalar(v2[:], sq[:], -1.0, 2.0, op0=ALU.mult, op1=ALU.add)
            gpp0 = constp.tile([D, BS], F32)
            nc.vector.tensor_mul(gpp0[:], ex[:], v2[:])
            gpp = constp.tile([D, BS], F32)
            nc.vector.tensor_scalar_mul(gpp[:], gpp0[:], INV_SQRT_2PI)

            # out = [g; 1].T @ [W2.T; b2]
            out_ps = spsum.tile([BS, D], F32, tag="sp")
            nc.tensor.matmul(out_ps[:], gta[:], w2ta, start=True, stop=True)
            out_sb = smallp.tile([BS, D], F32, tag="outsb")
            nc.vector.tensor_copy(out_sb[:], out_ps[:])
            nc.scalar.dma_start(out_d[:], out_sb[:])

            # 128-partition source for the stage-acquisition dummy copies
            dummy = constp.tile([P, 1], F32)
            nc.vector.memset(dummy[:], 0.0)

            # ---- M[k, j, l] = W1[k,j] * W1[k,l]  (rhs for hessian matmuls) ----
            # outer product via step-0 broadcast access patterns, on the
            # otherwise-idle GpSimd engine (keeps the vector engine free for
            # the forward chain and block 0's scales), split into quarters so
            # block 0's first chunks can start before the whole 4MB table is
            # built (deps are range-tracked)
            m_t = mpool.tile([D, D, D], F32R)
            MQ = 4
            for q in range(MQ):
                j0, j1 = q * D // MQ, (q + 1) * D // MQ
                w1_bj = bass.AP(
                    w1.tensor, w1.offset + j0, [w1.ap[0], [1, j1 - j0], [0, D]]
                )
                w1_bl = bass.AP(w1.tensor, w1.offset, [w1.ap[0], [0, j1 - j0], [1, D]])
                nc.gpsimd.tensor_mul(m_t[:, j0:j1], w1_bj, w1_bl)

            # ---- per-block jacobian + hessian over flat rows r=(b,i) ----
            for blk in range(NBLK):
                r0 = blk * P

                def build_scaled(tile_ap, gvec):
                    # tile[:, c] = W2.T[:, i(r0+c)] * gvec[:, b(r0+c)]
                    # rows past ROWS are padding: clamped b -> garbage values,
                    # written to the padded DRAM tail and never read
                    c = 0
                    while c < P:
                        b_c, i_c = divmod(r0 + c, D)
                        b_c = min(b_c, BS - 1)
                        n_c = min(D - i_c, P - c)
                        nc.vector.tensor_scalar_mul(
                            tile_ap[:, c : c + n_c],
                            w2t[:, i_c : i_c + n_c],
                            gvec[:, b_c : b_c + 1],
                        )
                        c += n_c

                # hessian rows r0..r0+P (float32r)
                hl = smallp.tile([D, P], F32R, tag="hl")
                build_scaled(hl, gpp)
                stage = stagep.tile([P, NCHUNK, CW], F32)
                # dummy first touch: absorbs the buffer-reuse (hess DMA of
                # block blk-3) wait so the real copies keep short wait lists
                nc.vector.tensor_copy(stage[:, 0, :1], dummy[:])
                SUB = NCHUNK // 4  # chunks per sub-DMA
                for c in range(NCHUNK):
                    h_ps = hpsum.tile([P, 512], F32)
                    jlo = c * (CW // D)  # start j of this chunk
                    nc.tensor.matmul(
                        h_ps[:, :CW],
                        hl[:],
                        m_t[:, jlo : jlo + CW // D, :],
                        start=True,
                        stop=True,
                    )
                    # split PSUM->SBUF copies between vector and scalar engines
                    if c % 2 == 1:
                        nc.scalar.copy(stage[:, c], h_ps[:, :CW])
                    else:
                        nc.vector.tensor_copy(stage[:, c], h_ps[:, :CW])
                    if c % SUB == SUB - 1:
                        # stream each quarter as soon as its copies land so
                        # the store DMA never waits for a whole block
                        c0 = c - (SUB - 1)
                        nc.sync.dma_start(
                            hess_d[r0 : r0 + P, c0 * CW : (c + 1) * CW],
                            stage[:, c0 : c + 1].rearrange("p a b -> p (a b)"),
                        )

                # jacobian rows r0..r0+P (exact fp32)
                jl = smallp.tile([D, P], F32, tag="jl")
                build_scaled(jl, gp)
                j_ps = spsum.tile([P, D], F32, tag="sp")
                nc.tensor.matmul(j_ps[:], jl[:], w1, start=True, stop=True)
                jstage = smallp.tile([P, D], F32, tag="jstage")
                nc.vector.tensor_copy(jstage[:], j_ps[:])
                nc.scalar.dma_start(jac_d[r0 : r0 + P, :], jstage[:])

    nc.finalize()
    return nc


_NC_CACHE = {}


def _get_nc():
    if "nc" not in _NC_CACHE:
        _NC_CACHE["nc"] = build_nc()
    return _NC_CACHE["nc"]


def _prep_inputs(x, W1, b1, W2, b2):
    x = np.asarray(x, dtype=np.float32)
    W1 = np.ascontiguousarray(np.asarray(W1, dtype=np.float32))
    b1 = np.asarray(b1, dtype=np.float32)
    W2 = np.asarray(W2, dtype=np.float32)
    b2 = np.asarray(b2, dtype=np.float32)

    w1pad = np.concatenate([W1, np.zeros((1, D), np.float32)], axis=0)  # (D+1, D)
    w1ta = np.concatenate([W1.T, b1[None, :]], axis=0)  # (D+1, D)
    w2ta = np.concatenate([W2.T, b2[None, :]], axis=0)  # (D+1, D)

    in_maps = []
    for c in range(NCORES):
        xs = x[c * BS : (c + 1) * BS]  # (BS, D)
        xta = np.concatenate([xs.T, np.ones((1, BS), np.float32)], axis=0)
        pk = np.ascontiguousarray(
            np.concatenate([xta, w1pad, w1ta, w2ta], axis=1)
        )  # (D+1, 316)
        in_maps.append({"pk": pk})
    return in_maps


def run(x, W1, b1, W2, b2, trace=False, **kw):
    nc = _get_nc()
    in_maps = _prep_inputs(x, W1, b1, W2, b2)
    res = run_bass_kernel_spmd(nc, in_maps, list(range(NCORES)), trace=trace, **kw)
    out = np.concatenate([r["out"] for r in res.results], axis=0)
    jac = np.concatenate(
        [r["jac"][:ROWS].reshape(BS, D, D) for r in res.results], axis=0
    )
    hess = np.concatenate(
        [r["hess"][:ROWS].reshape(BS, D, D, D) for r in res.results], axis=0
    )
    return (out, jac, hess), res


def kernel(x, W1, b1, W2, b2):
    (out, jac, hess), _ = run(x, W1, b1, W2, b2)
    return (out, jac, hess)


# revision 38
# speedup vs baseline: 1.1491x; 1.1406x over previous
"""Trainium2 Bass kernel: FFN forward + per-sample Jacobian + Hessian.

Reference computation (per sample xi of width D=100):
    h   = gelu(xi @ W1.T + b1)            (exact erf GELU)
    y   = h @ W2.T + b2
    out[b]  = y                               (B, D)
    jac[b]  = W2 @ diag(g'(pre_b)) @ W1       (B, D, D)
    hess[b,i,j,l] = sum_k W2[i,k] g''(pre_b k) W1[k,j] W1[k,l]   (B, D, D, D)

Strategy: pure data parallel over the batch (B=128 -> 16 per core, 8 cores),
weights replicated. Per core the outputs are treated as a flat row space
r = (b, i) of BS*D = 1600 rows:
    jac_flat[r, j]   = sum_k (W2[i,k] g'(pre_b,k))  W1[k,j]
    hess_flat[r, jl] = sum_k (W2[i,k] g''(pre_b,k)) M[k, jl]
with M[k, j*D+l] = W1[k,j]*W1[k,l] built on-chip once. Rows are processed in
blocks of 128 so every matmul output, PSUM->SBUF copy, and DMA store uses all
128 partitions — a 100-partition store stream only reaches ~176 GB/s of HBM
write bandwidth while 128 partitions reach ~373 GB/s (measured), and the
512 MB hessian write is the roofline. Each 128-row block spans two samples, so
its stationary operand is built with two per-partition scales of W2.T.

The hessian matmuls run in float32r (tf32-like, 1 cycle/row; plain fp32 costs
4) — hess rel err ~1.5e-4. Forward output and jacobian stay exact fp32.
"""

import numpy as np

import concourse.bass as bass
import concourse.bacc as bacc
import concourse.mybir as mybir
from concourse.tile import TileContext
from concourse.bass_utils import run_bass_kernel_spmd

B, D = 128, 100
NCORES = 8
BS = B // NCORES  # 16 samples per core
DD = D * D
ROWS = BS * D  # 1600 output rows per core
P = 128  # rows per block
NBLK = (ROWS + P - 1) // P  # 13
RPAD = NBLK * P  # 1664: outputs padded to full blocks; host slices [:ROWS]

F32 = mybir.dt.float32
F32R = mybir.dt.float32r
ACT = mybir.ActivationFunctionType
ALU = mybir.AluOpType

INV_SQRT2 = 0.7071067811865476
INV_SQRT_2PI = 0.3989422804014327

NCHUNK = 20  # hess free-dim chunks per block
CW = 500  # chunk width (jl columns per matmul; 500*4B fits one PSUM bank)

# packed input layout: [xta | w1ta | w1 | w2ta] along the free dim; the
# first two (needed by the forward matmul) load on one DMA ring while the
# rest load in parallel on the other
PK_X = 0
PK_W1TA = PK_X + BS
PK_W1 = PK_W1TA + D
PK_W2TA = PK_W1 + D
PK_W = PK_W2TA + D  # 316


def build_nc():
    # Bacc (not raw Bass): its finalize() runs compile() passes that split
    # multi-semaphore waits into EventSemaphore instructions (TRN2 allows at
    # most one sync wait per regular instruction).
    nc = bacc.Bacc()

    pk_d = nc.dram_tensor("pk", [D + 1, PK_W], F32, kind="ExternalInput")

    out_d = nc.dram_tensor("out", [BS, D], F32, kind="ExternalOutput")
    # jac stored (p, blk, j): one DMA with a single contiguous descriptor per
    # partition; the host reorders rows (row r = blk*P + p)
    jac_d = nc.dram_tensor("jac", [P, NBLK, D], F32, kind="ExternalOutput")
    hess_d = nc.dram_tensor("hess", [RPAD, DD], F32, kind="ExternalOutput")

    with TileContext(nc) as tc:
        with (
            tc.tile_pool(name="const", bufs=1) as constp,
            tc.tile_pool(name="mpool", bufs=1) as mpool,
            tc.tile_pool(name="stage", bufs=3) as stagep,
            tc.tile_pool(name="small", bufs=2) as smallp,
            tc.tile_pool(name="spsum", bufs=2, space="PSUM") as spsum,
            tc.tile_pool(name="hpsum", bufs=6, space="PSUM") as hpsum,
        ):
            # ---- load inputs: two parallel DMAs on separate HWDGE rings ----
            pk = constp.tile([D + 1, PK_W], F32)
            nc.sync.dma_start(pk[:, :PK_W1], pk_d[:, :PK_W1])
            nc.scalar.dma_start(pk[:, PK_W1:], pk_d[:, PK_W1:])
            xta = pk[:, PK_X:PK_W1TA]  # (D+1, BS)
            w1ta = pk[:, PK_W1TA:PK_W1]  # (D+1, D)
            w1 = pk[:D, PK_W1:PK_W2TA]  # (D, D)
            w2ta = pk[:, PK_W2TA:PK_W]  # (D+1, D)
            w2t = pk[:D, PK_W2TA:PK_W]  # (D, D)

            # prefetch both scalar-engine activation tables (erf, exp) while
            # the input DMA is still in flight; each table load costs ~1.3us
            # and would otherwise sit on the forward critical path
            warm = constp.tile([P, 1], F32)
            nc.vector.memset(warm[:], 0.0)
            nc.scalar.activation(warm[:], warm[:], ACT.Erf)
            nc.scalar.activation(warm[:], warm[:], ACT.Exp)

            # ---- forward: pre_T = [W1.T; b1].T @ [x.T; 1] ----
            pre_ps = spsum.tile([D, BS], F32, tag="sp")
            nc.tensor.matmul(pre_ps[:], w1ta, xta, start=True, stop=True)
            pre = constp.tile([D, BS], F32)
            nc.vector.tensor_copy(pre[:], pre_ps[:])

            # Phi = 0.5*(1 + erf(pre/sqrt(2)))
            erf_t = constp.tile([D, BS], F32)
            nc.scalar.activation(erf_t[:], pre[:], ACT.Erf, scale=INV_SQRT2)
            phi_t = constp.tile([D, BS], F32)
            nc.vector.tensor_scalar(
                phi_t[:], erf_t[:], 0.5, 0.5, op0=ALU.mult, op1=ALU.add
            )
            # sq = pre^2 ; ex = exp(-pre^2/2)
            sq = constp.tile([D, BS], F32)
            nc.scalar.activation(sq[:], pre[:], ACT.Square)
            ex = constp.tile([D, BS], F32)
            nc.scalar.activation(ex[:], sq[:], ACT.Exp, scale=-0.5)

            # g = pre * Phi, with ones row appended for the bias trick.
            # Compute-engine writes must start at partition 0/32/64/96, so
            # memset rows 96..100 to 1 first; the mul then overwrites 96..99.
            gta = constp.tile([D + 1, BS], F32)
            nc.vector.memset(gta[96 : D + 1, :], 1.0)
            nc.vector.tensor_mul(gta[:D, :], pre[:], phi_t[:])

            # gp = Phi + c * pre * ex          (c = 1/sqrt(2*pi))
            t1 = constp.tile([D, BS], F32)
            nc.vector.tensor_mul(t1[:], pre[:], ex[:])
            t1s = constp.tile([D, BS], F32)
            nc.vector.tensor_scalar_mul(t1s[:], t1[:], INV_SQRT_2PI)
            gp = constp.tile([D, BS], F32)
            nc.vector.tensor_add(gp[:], phi_t[:], t1s[:])

            # gpp = c * ex * (2 - sq)
            v2 = constp.tile([D, BS], F32)
            nc.vector.tensor_scalar(v2[:], sq[:], -1.0, 2.0, op0=ALU.mult, op1=ALU.add)
            gpp0 = constp.tile([D, BS], F32)
            nc.vector.tensor_mul(gpp0[:], ex[:], v2[:])
            gpp = constp.tile([D, BS], F32)
            nc.vector.tensor_scalar_mul(gpp[:], gpp0[:], INV_SQRT_2PI)

            # out = [g; 1].T @ [W2.T; b2]
            out_ps = spsum.tile([BS, D], F32, tag="sp")
            nc.tensor.matmul(out_ps[:], gta[:], w2ta, start=True, stop=True)
            out_sb = smallp.tile([BS, D], F32, tag="outsb")
            nc.vector.tensor_copy(out_sb[:], out_ps[:])
            nc.scalar.dma_start(out_d[:], out_sb[:])

            # 128-partition source for the stage-acquisition dummy copies
            dummy = constp.tile([P, 1], F32)
            nc.vector.memset(dummy[:], 0.0)

            # ---- M[k, j, l] = W1[k,j] * W1[k,l]  (rhs for hessian matmuls) ----
            # outer product via step-0 broadcast access patterns, on the
            # otherwise-idle GpSimd engine (keeps the vector engine free for
            # the forward chain and block 0's scales), split into quarters so
            # block 0's first chunks can start before the whole 4MB table is
            # built (deps are range-tracked)
            m_t = mpool.tile([D, D, D], F32R)
            MQ = 4
            for q in range(MQ):
                j0, j1 = q * D // MQ, (q + 1) * D // MQ
                w1_bj = bass.AP(
                    w1.tensor, w1.offset + j0, [w1.ap[0], [1, j1 - j0], [0, D]]
                )
                w1_bl = bass.AP(w1.tensor, w1.offset, [w1.ap[0], [0, j1 - j0], [1, D]])
                eng = nc.vector if q < 2 else nc.gpsimd
                eng.tensor_mul(m_t[:, j0:j1], w1_bj, w1_bl)

            # persistent jacobian buffer: one efficient DMA at the end
            jac_all = constp.tile([P, NBLK, D], F32)

            # ---- per-block jacobian + hessian over flat rows r=(b,i) ----
            for blk in range(NBLK):
                r0 = blk * P

                def build_scaled(tile_ap, gvec):
                    # tile[:, c] = W2.T[:, i(r0+c)] * gvec[:, b(r0+c)]
                    # rows past ROWS are padding: clamped b -> garbage values,
                    # written to the padded DRAM tail and never read
                    c = 0
                    while c < P:
                        b_c, i_c = divmod(r0 + c, D)
                        b_c = min(b_c, BS - 1)
                        n_c = min(D - i_c, P - c)
                        nc.vector.tensor_scalar_mul(
                            tile_ap[:, c : c + n_c],
                            w2t[:, i_c : i_c + n_c],
                            gvec[:, b_c : b_c + 1],
                        )
                        c += n_c

                # hessian rows r0..r0+P (float32r)
                hl = smallp.tile([D, P], F32R, tag="hl")
                build_scaled(hl, gpp)
                stage = stagep.tile([P, NCHUNK, CW], F32)
                # dummy first touch: absorbs the buffer-reuse (hess DMA of
                # block blk-3) wait so the real copies keep short wait lists
                nc.vector.tensor_copy(stage[:, 0, :1], dummy[:])
                # block 0 streams per-quarter so the store DMA starts early;
                # later blocks use one full-block DMA (fewer, bigger bursts
                # sustain a higher store rate)
                SUB = NCHUNK // 4 if blk == 0 else NCHUNK
                for c in range(NCHUNK):
                    h_ps = hpsum.tile([P, 512], F32)
                    jlo = c * (CW // D)  # start j of this chunk
                    nc.tensor.matmul(
                        h_ps[:, :CW],
                        hl[:],
                        m_t[:, jlo : jlo + CW // D, :],
                        start=True,
                        stop=True,
                    )
                    # split PSUM->SBUF copies between vector and scalar engines
                    if c % 2 == 1:
                        nc.scalar.copy(stage[:, c], h_ps[:, :CW])
                    else:
                        nc.vector.tensor_copy(stage[:, c], h_ps[:, :CW])
                    if c % SUB == SUB - 1:
                        c0 = c - (SUB - 1)
                        nc.sync.dma_start(
                            hess_d[r0 : r0 + P, c0 * CW : (c + 1) * CW],
                            stage[:, c0 : c + 1].rearrange("p a b -> p (a b)"),
                        )

                # jacobian rows r0..r0+P (exact fp32)
                jl = smallp.tile([D, P], F32, tag="jl")
                build_scaled(jl, gp)
                j_ps = spsum.tile([P, D], F32, tag="sp")
                nc.tensor.matmul(j_ps[:], jl[:], w1, start=True, stop=True)
                nc.vector.tensor_copy(jac_all[:, blk, :], j_ps[:])

            nc.scalar.dma_start(jac_d[:], jac_all[:])

    nc.finalize()
    return nc


_NC_CACHE = {}


def _get_nc():
    if "nc" not in _NC_CACHE:
        _NC_CACHE["nc"] = build_nc()
    return _NC_CACHE["nc"]


def _prep_inputs(x, W1, b1, W2, b2):
    x = np.asarray(x, dtype=np.float32)
    W1 = np.ascontiguousarray(np.asarray(W1, dtype=np.float32))
    b1 = np.asarray(b1, dtype=np.float32)
    W2 = np.asarray(W2, dtype=np.float32)
    b2 = np.asarray(b2, dtype=np.float32)

    w1pad = np.concatenate([W1, np.zeros((1, D), np.float32)], axis=0)  # (D+1, D)
    w1ta = np.concatenate([W1.T, b1[None, :]], axis=0)  # (D+1, D)
    w2ta = np.concatenate([W2.T, b2[None, :]], axis=0)  # (D+1, D)

    in_maps = []
    for c in range(NCORES):
        xs = x[c * BS : (c + 1) * BS]  # (BS, D)
        xta = np.concatenate([xs.T, np.ones((1, BS), np.float32)], axis=0)
        pk = np.ascontiguousarray(
            np.concatenate([xta, w1ta, w1pad, w2ta], axis=1)
        )  # (D+1, 316)
        in_maps.append({"pk": pk})
    return in_maps


def run(x, W1, b1, W2, b2, trace=False, **kw):
    nc = _get_nc()
    in_maps = _prep_inputs(x, W1, b1, W2, b2)
    res = run_bass_kernel_spmd(nc, in_maps, list(range(NCORES)), trace=trace, **kw)
    out = np.concatenate([r["out"] for r in res.results], axis=0)
    jac = np.concatenate(
        [
            # (p, blk, j) -> flat rows r = blk*P + p
            r["jac"].transpose(1, 0, 2).reshape(RPAD, D)[:ROWS].reshape(BS, D, D)
            for r in res.results
        ],
        axis=0,
    )
    hess = np.concatenate(
        [r["hess"][:ROWS].reshape(BS, D, D, D) for r in res.results], axis=0
    )
    return (out, jac, hess), res


def kernel(x, W1, b1, W2, b2):
    (out, jac, hess), _ = run(x, W1, b1, W2, b2)
    return (out, jac, hess)


# revision 41
# speedup vs baseline: 1.2320x; 1.0721x over previous
"""Trainium2 Bass kernel: FFN forward + per-sample Jacobian + Hessian.

Reference computation (per sample xi of width D=100):
    h   = gelu(xi @ W1.T + b1)            (exact erf GELU)
    y   = h @ W2.T + b2
    out[b]  = y                               (B, D)
    jac[b]  = W2 @ diag(g'(pre_b)) @ W1       (B, D, D)
    hess[b,i,j,l] = sum_k W2[i,k] g''(pre_b k) W1[k,j] W1[k,l]   (B, D, D, D)

Strategy: pure data parallel over the batch (B=128 -> 16 per core, 8 cores),
weights replicated. Per core the outputs are treated as a flat row space
r = (b, i) of BS*D = 1600 rows:
    jac_flat[r, j]   = sum_k (W2[i,k] g'(pre_b,k))  W1[k,j]
    hess_flat[r, jl] = sum_k (W2[i,k] g''(pre_b,k)) M[k, jl]
with M[k, j*D+l] = W1[k,j]*W1[k,l] built on-chip once. Rows are processed in
blocks of 128 so every matmul output, PSUM->SBUF copy, and DMA store uses all
128 partitions — a 100-partition store stream only reaches ~176 GB/s of HBM
write bandwidth while 128 partitions reach ~373 GB/s (measured), and the
512 MB hessian write is the roofline. Each 128-row block spans two samples, so
its stationary operand is built with two per-partition scales of W2.T.

The hessian matmuls run in float32r (tf32-like, 1 cycle/row; plain fp32 costs
4) — hess rel err ~1.5e-4. Forward output and jacobian stay exact fp32.
"""

import numpy as np

import concourse.bass as bass
import concourse.bacc as bacc
import concourse.mybir as mybir
from concourse.tile import TileContext
from concourse.bass_utils import run_bass_kernel_spmd

B, D = 128, 100
NCORES = 8
BS = B // NCORES  # 16 samples per core
DD = D * D
ROWS = BS * D  # 1600 output rows per core
P = 128  # rows per block
NBLK = (ROWS + P - 1) // P  # 13
RPAD = NBLK * P  # 1664: outputs padded to full blocks; host slices [:ROWS]

F32 = mybir.dt.float32
F32R = mybir.dt.float32r
ACT = mybir.ActivationFunctionType
ALU = mybir.AluOpType

INV_SQRT2 = 0.7071067811865476
INV_SQRT_2PI = 0.3989422804014327

NCHUNK = 20  # hess free-dim chunks per block
CW = 500  # chunk width (jl columns per matmul; 500*4B fits one PSUM bank)

# packed input layout: [xta | w1ta | w1 | w2ta] along the free dim; the
# first two (needed by the forward matmul) load on one DMA ring while the
# rest load in parallel on the other
PK_X = 0
PK_W1TA = PK_X + BS
PK_W1 = PK_W1TA + D
PK_W2TA = PK_W1 + D
PK_W = PK_W2TA + D  # 316


def build_nc():
    # Bacc (not raw Bass): its finalize() runs compile() passes that split
    # multi-semaphore waits into EventSemaphore instructions (TRN2 allows at
    # most one sync wait per regular instruction).
    nc = bacc.Bacc()

    pk_d = nc.dram_tensor("pk", [D + 1, PK_W], F32, kind="ExternalInput")

    out_d = nc.dram_tensor("out", [BS, D], F32, kind="ExternalOutput")
    # jac stored (p, blk, j): one DMA with a single contiguous descriptor per
    # partition; the host reorders rows (row r = blk*P + p)
    jac_d = nc.dram_tensor("jac", [P, NBLK, D], F32, kind="ExternalOutput")
    hess_d = nc.dram_tensor("hess", [RPAD, DD], F32, kind="ExternalOutput")

    with TileContext(nc) as tc:
        with (
            tc.tile_pool(name="const", bufs=1) as constp,
            tc.tile_pool(name="mpool", bufs=1) as mpool,
            tc.tile_pool(name="stage", bufs=3) as stagep,
            tc.tile_pool(name="small", bufs=2) as smallp,
            tc.tile_pool(name="spsum", bufs=2, space="PSUM") as spsum,
            tc.tile_pool(name="hpsum", bufs=6, space="PSUM") as hpsum,
        ):
            # ---- load inputs: two parallel DMAs on separate HWDGE rings ----
            pk = constp.tile([D + 1, PK_W], F32)
            nc.sync.dma_start(pk[:, :PK_W1], pk_d[:, :PK_W1])
            nc.scalar.dma_start(pk[:, PK_W1:], pk_d[:, PK_W1:])
            xta = pk[:, PK_X:PK_W1TA]  # (D+1, BS)
            w1ta = pk[:, PK_W1TA:PK_W1]  # (D+1, D)
            w1 = pk[:D, PK_W1:PK_W2TA]  # (D, D)
            w2ta = pk[:, PK_W2TA:PK_W]  # (D+1, D)
            w2t = pk[:D, PK_W2TA:PK_W]  # (D, D)

            # prefetch the erf activation table while the input DMA is in
            # flight (the load costs ~1.3us and would sit on the forward
            # critical path; Erf is the first ACT function used, so the
            # warmed table is still resident)
            warm = constp.tile([P, 1], F32)
            nc.vector.memset(warm[:], 0.0)
            nc.scalar.activation(warm[:], warm[:], ACT.Erf)

            # ---- forward: pre_T = [W1.T; b1].T @ [x.T; 1] ----
            pre_ps = spsum.tile([D, BS], F32, tag="sp")
            nc.tensor.matmul(pre_ps[:], w1ta, xta, start=True, stop=True)
            pre = constp.tile([D, BS], F32)
            nc.vector.tensor_copy(pre[:], pre_ps[:])

            # Phi = 0.5*(1 + erf(pre/sqrt(2)))  (erf reads PSUM directly,
            # in parallel with the copy above)
            erf_t = constp.tile([D, BS], F32)
            nc.scalar.activation(erf_t[:], pre_ps[:], ACT.Erf, scale=INV_SQRT2)
            phi_t = constp.tile([D, BS], F32)
            nc.vector.tensor_scalar(
                phi_t[:], erf_t[:], 0.5, 0.5, op0=ALU.mult, op1=ALU.add
            )
            # sq = pre^2 on the vector engine (keeps Square's table off the
            # scalar engine — fewer ACT table switches) ; ex = exp(-pre^2/2)
            sq = constp.tile([D, BS], F32)
            nc.vector.tensor_mul(sq[:], pre[:], pre[:])
            ex = constp.tile([D, BS], F32)
            nc.scalar.activation(ex[:], sq[:], ACT.Exp, scale=-0.5)

            # g = pre * Phi, with ones row appended for the bias trick.
            # Compute-engine writes must start at partition 0/32/64/96, so
            # memset rows 96..100 to 1 first; the mul then overwrites 96..99.
            gta = constp.tile([D + 1, BS], F32)
            nc.vector.memset(gta[96 : D + 1, :], 1.0)
            nc.vector.tensor_mul(gta[:D, :], pre[:], phi_t[:])

            # gp = Phi + c * pre * ex          (c = 1/sqrt(2*pi))
            t1 = constp.tile([D, BS], F32)
            nc.vector.tensor_mul(t1[:], pre[:], ex[:])
            t1s = constp.tile([D, BS], F32)
            nc.vector.tensor_scalar_mul(t1s[:], t1[:], INV_SQRT_2PI)
            gp = constp.tile([D, BS], F32)
            nc.vector.tensor_add(gp[:], phi_t[:], t1s[:])

            # gpp = c * ex * (2 - sq)
            v2 = constp.tile([D, BS], F32)
            nc.vector.tensor_scalar(v2[:], sq[:], -1.0, 2.0, op0=ALU.mult, op1=ALU.add)
            gpp0 = constp.tile([D, BS], F32)
            nc.vector.tensor_mul(gpp0[:], ex[:], v2[:])
            gpp = constp.tile([D, BS], F32)
            nc.vector.tensor_scalar_mul(gpp[:], gpp0[:], INV_SQRT_2PI)

            # out = [g; 1].T @ [W2.T; b2]
            out_ps = spsum.tile([BS, D], F32, tag="sp")
            nc.tensor.matmul(out_ps[:], gta[:], w2ta, start=True, stop=True)
            out_sb = smallp.tile([BS, D], F32, tag="outsb")
            nc.vector.tensor_copy(out_sb[:], out_ps[:])
            nc.scalar.dma_start(out_d[:], out_sb[:])

            # 128-partition source for the stage-acquisition dummy copies
            dummy = constp.tile([P, 1], F32)
            nc.vector.memset(dummy[:], 0.0)

            # ---- M[k, j, l] = W1[k,j] * W1[k,l]  (rhs for hessian matmuls) ----
            # outer product via step-0 broadcast access patterns, on the
            # otherwise-idle GpSimd engine (keeps the vector engine free for
            # the forward chain and block 0's scales), split into quarters so
            # block 0's first chunks can start before the whole 4MB table is
            # built (deps are range-tracked)
            # splits at even element offsets (odd f32 offsets drop the DVE
            # 2x fp32 mode); GpSimd is ~2.5x slower so it gets the smallest
            # piece, running concurrently with the two DVE pieces
            m_t = mpool.tile([D, D, D], F32R)
            for j0, j1, eng in (
                (0, 36, nc.vector),
                (36, 72, nc.vector),
                (72, 100, nc.gpsimd),
            ):
                w1_bj = bass.AP(
                    w1.tensor, w1.offset + j0, [w1.ap[0], [1, j1 - j0], [0, D]]
                )
                w1_bl = bass.AP(w1.tensor, w1.offset, [w1.ap[0], [0, j1 - j0], [1, D]])
                eng.tensor_mul(m_t[:, j0:j1], w1_bj, w1_bl)

            # persistent jacobian buffer: one efficient DMA at the end
            jac_all = constp.tile([P, NBLK, D], F32)

            # ---- per-block jacobian + hessian over flat rows r=(b,i) ----
            for blk in range(NBLK):
                r0 = blk * P

                def build_scaled(tile_ap, gvec):
                    # tile[:, c] = W2.T[:, i(r0+c)] * gvec[:, b(r0+c)]
                    # rows past ROWS are padding: clamped b -> garbage values,
                    # written to the padded DRAM tail and never read
                    c = 0
                    while c < P:
                        b_c, i_c = divmod(r0 + c, D)
                        b_c = min(b_c, BS - 1)
                        n_c = min(D - i_c, P - c)
                        nc.vector.tensor_scalar_mul(
                            tile_ap[:, c : c + n_c],
                            w2t[:, i_c : i_c + n_c],
                            gvec[:, b_c : b_c + 1],
                        )
                        c += n_c

                # hessian rows r0..r0+P (float32r)
                hl = smallp.tile([D, P], F32R, tag="hl")
                build_scaled(hl, gpp)
                stage = stagep.tile([P, NCHUNK, CW], F32)
                # dummy first touch: absorbs the buffer-reuse (hess DMA of
                # block blk-3) wait so the real copies keep short wait lists
                nc.vector.tensor_copy(stage[:, 0, :1], dummy[:])
                # block 0 streams per-quarter so the store DMA starts early;
                # later blocks use one full-block DMA (fewer, bigger bursts
                # sustain a higher store rate)
                SUB = NCHUNK // 4 if blk == 0 else NCHUNK
                for c in range(NCHUNK):
                    h_ps = hpsum.tile([P, 512], F32)
                    jlo = c * (CW // D)  # start j of this chunk
                    nc.tensor.matmul(
                        h_ps[:, :CW],
                        hl[:],
                        m_t[:, jlo : jlo + CW // D, :],
                        start=True,
                        stop=True,
                    )
                    # split PSUM->SBUF copies between vector and scalar engines
                    if c % 2 == 1:
                        nc.scalar.copy(stage[:, c], h_ps[:, :CW])
                    else:
                        nc.vector.tensor_copy(stage[:, c], h_ps[:, :CW])
                    if c % SUB == SUB - 1:
                        c0 = c - (SUB - 1)
                        # last block only has 64 real rows; don't burn store
                        # bandwidth writing padding
                        rdma = min(P, ROWS - r0)
                        nc.sync.dma_start(
                            hess_d[r0 : r0 + rdma, c0 * CW : (c + 1) * CW],
                            stage[:rdma, c0 : c + 1].rearrange("p a b -> p (a b)"),
                        )

                # jacobian rows r0..r0+P (exact fp32)
                jl = smallp.tile([D, P], F32, tag="jl")
                build_scaled(jl, gp)
                j_ps = spsum.tile([P, D], F32, tag="sp")
                nc.tensor.matmul(j_ps[:], jl[:], w1, start=True, stop=True)
                nc.vector.tensor_copy(jac_all[:, blk, :], j_ps[:])

            nc.scalar.dma_start(jac_d[:], jac_all[:])

    nc.finalize()
    return nc


_NC_CACHE = {}


def _get_nc():
    if "nc" not in _NC_CACHE:
        _NC_CACHE["nc"] = build_nc()
    return _NC_CACHE["nc"]


def _prep_inputs(x, W1, b1, W2, b2):
    x = np.asarray(x, dtype=np.float32)
    W1 = np.ascontiguousarray(np.asarray(W1, dtype=np.float32))
    b1 = np.asarray(b1, dtype=np.float32)
    W2 = np.asarray(W2, dtype=np.float32)
    b2 = np.asarray(b2, dtype=np.float32)

    w1pad = np.concatenate([W1, np.zeros((1, D), np.float32)], axis=0)  # (D+1, D)
    w1ta = np.concatenate([W1.T, b1[None, :]], axis=0)  # (D+1, D)
    w2ta = np.concatenate([W2.T, b2[None, :]], axis=0)  # (D+1, D)

    in_maps = []
    for c in range(NCORES):
        xs = x[c * BS : (c + 1) * BS]  # (BS, D)
        xta = np.concatenate([xs.T, np.ones((1, BS), np.float32)], axis=0)
        pk = np.ascontiguousarray(
            np.concatenate([xta, w1ta, w1pad, w2ta], axis=1)
        )  # (D+1, 316)
        in_maps.append({"pk": pk})
    return in_maps


def run(x, W1, b1, W2, b2, trace=False, **kw):
    nc = _get_nc()
    in_maps = _prep_inputs(x, W1, b1, W2, b2)
    res = run_bass_kernel_spmd(nc, in_maps, list(range(NCORES)), trace=trace, **kw)
    out = np.concatenate([r["out"] for r in res.results], axis=0)
    jac = np.concatenate(
        [
            # (p, blk, j) -> flat rows r = blk*P + p
            r["jac"].transpose(1, 0, 2).reshape(RPAD, D)[:ROWS].reshape(BS, D, D)
            for r in res.results
        ],
        axis=0,
    )
    hess = np.concatenate(
        [r["hess"][:ROWS].reshape(BS, D, D, D) for r in res.results], axis=0
    )
    return (out, jac, hess), res


def kernel(x, W1, b1, W2, b2):
    (out, jac, hess), _ = run(x, W1, b1, W2, b2)
    return (out, jac, hess)
